# revision 8
# baseline (speedup 1.0000x reference)
"""Causal single-head attention (nn_AttentionHead) on 8 TRN2 NeuronCores.

Self-contained: kernel(**inputs) takes the full fp32 inputs and returns the
full [4, 4096, 64] output, distributing work across 8 cores internally.

Sharding: 8 cores = 4 batches x 2 key-parity shards. Core (b, h) computes,
for ALL 4096 queries of batch b (in parity-permuted order), unnormalized
flash-attention partials (numerator [64] + denominator) over the keys in
128-row blocks of parity h; the host sums the two partials per batch and
normalizes (softmax without max-subtraction is exact here; scores are O(1)).

v3 design (measured lineage: 84us -> 60.6us -> this):
  - bf16 datapath: x, weights, K/V/Q, exp output all bf16 (fp32 PSUM accum).
  - Explicit tile_position packing (measured ~1.9x on HW):
      * score matmuls: K=64 pairs row-tiled at (0,0)/(64,0)
      * q-pass: M=64 chunk pairs col-tiled at (0,0)/(0,64), single PSUM bank
  - Causal trimming: near-diagonal score/PV matmuls stream only the
    not-fully-masked query range; per-diagonal-block [128,128] masks
    (triangle first half, parity scalar second half).
  - V to natural layout via xbar DMA transpose (contiguous dest + DVE
    interleave copy).
  - Cross-rep software pipelining: the projection work (kv/q/vp) of rep r+1
    is emitted interleaved into the attention chunks of rep r, so the ACT
    engine (exp) never drains and PE stays dense. All rep-state double
    buffered (xqt, kvt, ktd, qt, vp).
  - exp on ACT only; PSUM->SBUF copies, biases, masks on DVE.
"""

import os
import sys
from contextlib import ExitStack

import numpy as np

for _p in ("/root/.axon_site/_ro/trn_rl_repo", "/opt/trn_rl_repo"):
    if os.path.isdir(_p) and _p not in sys.path:
        sys.path.append(_p)

import concourse.bacc as bacc
import concourse.tile as tile
from concourse import mybir

F32 = mybir.dt.float32
BF16 = mybir.dt.bfloat16

B, T, C, H = 4, 4096, 1024, 64
KT = C // 128  # contraction tiles
NKEY = T // 2  # keys per core
NKT = NKEY // 128  # key tiles per core
NQC = T // 512  # query chunks
NB = T // 128  # 128-row blocks


# ---------------------------------------------------------------- device ----
def build(reps: int = 1):
    nc = bacc.Bacc("TRN2", target_bir_lowering=False, debug=False)

    xq = nc.dram_tensor("xq", [C, T], BF16, kind="ExternalInput")
    wq = nc.dram_tensor("wq", [C, H], BF16, kind="ExternalInput")
    wkv = nc.dram_tensor("wkv", [C, 128], BF16, kind="ExternalInput")  # [Wk|Wv]
    bq8 = nc.dram_tensor("bq8", [H, 1], F32, kind="ExternalInput")  # bq/8
    bkv = nc.dram_tensor("bkv", [128, 1], F32, kind="ExternalInput")  # [bk;0]
    hmt = nc.dram_tensor("hmt", [128, 1], F32, kind="ExternalInput")  # 1-h
    tmask = nc.dram_tensor("tmask", [128, 128], BF16, kind="ExternalInput")

    outp = nc.dram_tensor("outp", [H + 1, T], F32, kind="ExternalOutput")

    with tile.TileContext(nc) as tc, ExitStack() as ctx:
        const = ctx.enter_context(tc.tile_pool(name="const", bufs=1))
        big = ctx.enter_context(tc.tile_pool(name="big", bufs=2))
        kvq = ctx.enter_context(tc.tile_pool(name="kvq", bufs=2))
        work = ctx.enter_context(tc.tile_pool(name="work", bufs=3))
        ps = ctx.enter_context(tc.tile_pool(name="ps", bufs=2, space="PSUM"))
        ps1 = ctx.enter_context(tc.tile_pool(name="ps1", bufs=1, space="PSUM"))

        wqt = const.tile([128, KT, H], BF16)
        nc.sync.dma_start(out=wqt, in_=wq.ap().rearrange("(k p) m -> p k m", p=128))
        wkvt = const.tile([128, KT, 128], BF16)
        nc.sync.dma_start(out=wkvt, in_=wkv.ap().rearrange("(k p) m -> p k m", p=128))
        bq8t = const.tile([H, 1], F32)
        nc.sync.dma_start(out=bq8t, in_=bq8.ap())
        bkvt = const.tile([128, 1], F32)
        nc.sync.dma_start(out=bkvt, in_=bkv.ap())
        hmtt = const.tile([128, 1], F32)
        nc.sync.dma_start(out=hmtt, in_=hmt.ap())
        tmt = const.tile([128, 128], BF16)
        nc.sync.dma_start(out=tmt, in_=tmask.ap())

        consts = (wqt, wkvt, bq8t, bkvt, hmtt, tmt)

        # rep-state tiles are pool-cycled (bufs=2); proj_items(r) emits the
        # work that builds rep r's state, attn_chunk(r, st, qc) consumes it.
        state = {}

        def proj_items(r):
            """Generator of emission thunks for rep r's projections."""
            xqt = big.tile([128, KT, T], BF16, tag="xqt")
            kvt = kvq.tile([128, NKEY], BF16, tag="kvt")
            ktd = kvq.tile([128, NKEY], BF16, tag="ktd")
            qt = kvq.tile([128, T], BF16, tag="qt")
            vp = kvq.tile([128, NKT, H + 1], BF16, tag="vp")
            vscr = kvq.tile([128, NKT, H], BF16, tag="vscr")
            st = (xqt, kvt, ktd, qt, vp)
            xq_r = xq.ap().rearrange("(k p) t -> p k t", p=128)

            def dma_x():
                for i in range(4):
                    sl = slice(1024 * i, 1024 * (i + 1))
                    nc.sync.dma_start(out=xqt[:, :, sl], in_=xq_r[:, :, sl])

            def kv_chunk(kc):
                sl = slice(512 * kc, 512 * (kc + 1))
                box = {}

                def fa():
                    pkv = ps1.tile([128, 1024], F32, tag="proj")
                    box["pkv"] = pkv
                    for k in range(KT // 2):
                        nc.tensor.matmul(
                            pkv[:, 0:512], lhsT=wkvt[:, k, :],
                            rhs=xqt[:, k, sl],
                            start=(k == 0), stop=False, skip_group_check=True,
                        )

                def fb():
                    pkv = box["pkv"]
                    for k in range(KT // 2, KT):
                        nc.tensor.matmul(
                            pkv[:, 0:512], lhsT=wkvt[:, k, :], rhs=xqt[:, k, sl],
                            start=False, stop=(k == KT - 1),
                            skip_group_check=True,
                        )
                    nc.vector.tensor_scalar_add(kvt[:, sl], pkv[:, 0:512], bkvt)
                    nc.sync.dma_start(out=ktd[64:128, sl], in_=kvt[0:64, sl])
                return [fa, fb]

            def vtrans():
                nc.sync.dma_start_transpose(vscr, kvt[64:128, :])
                nc.vector.tensor_copy(vp[:, :, 0:H], vscr)
                nc.vector.memset(vp[:, :, H], 1.0)

            def q_pair(qp):
                slA = slice(1024 * qp, 1024 * qp + 512)
                slB = slice(1024 * qp + 512, 1024 * qp + 1024)
                box = {}

                def emit_k(pq, k0, k1):
                    for k in range(k0, k1):
                        nc.tensor.matmul(
                            pq[0:64, 0:512], lhsT=wqt[:, k, :], rhs=xqt[:, k, slA],
                            start=(k == 0), stop=(k == KT - 1),
                            tile_position=(0, 0), skip_group_check=True,
                        )
                        nc.tensor.matmul(
                            pq[64:128, 512:1024], lhsT=wqt[:, k, :],
                            rhs=xqt[:, k, slB],
                            start=(k == 0), stop=(k == KT - 1),
                            tile_position=(0, 64), skip_group_check=True,
                        )

                def fa():
                    pq = ps1.tile([128, 1024], F32, tag="proj")
                    box["pq"] = pq
                    emit_k(pq, 0, KT // 2)

                def fb():
                    pq = box["pq"]
                    emit_k(pq, KT // 2, KT)
                    nc.vector.tensor_scalar(
                        qt[0:H, slA], pq[0:64, 0:512], 0.125, bq8t,
                        op0=mybir.AluOpType.mult, op1=mybir.AluOpType.add,
                    )
                    nc.vector.tensor_scalar(
                        qt[0:H, slB], pq[64:128, 512:1024], 0.125, bq8t,
                        op0=mybir.AluOpType.mult, op1=mybir.AluOpType.add,
                    )
                    nc.sync.dma_start(
                        out=qt[64:128, 1024 * qp : 1024 * qp + 1024],
                        in_=qt[0:64, 1024 * qp : 1024 * qp + 1024],
                    )
                return [fa, fb]

            items = [dma_x]
            for kc in range(4):
                items += kv_chunk(kc)
            items.append(vtrans)
            for qp in range(4):
                items += q_pair(qp)
            return st, items

        def attn_chunk(st, qc, feeder):
            xqt, kvt, ktd, qt, vp = st
            qsl = slice(512 * qc, 512 * (qc + 1))
            first_half = qc < NQC // 2
            cc = qc if first_half else qc - NQC // 2
            n_k = 4 * cc + 4
            acc = ps.tile([H + 1, 512], F32, tag="acc")

            def score_pair(jA, nA, oA, jB, nB, oB, sp):
                nc.tensor.matmul(
                    sp[:, 0:nA],
                    lhsT=kvt[0:64, 128 * jA : 128 * (jA + 1)],
                    rhs=qt[0:64, 512 * qc + oA : 512 * qc + oA + nA],
                    start=True, stop=True, tile_position=(0, 0),
                )
                nc.tensor.matmul(
                    sp[:, 512 : 512 + nB],
                    lhsT=ktd[64:128, 128 * jB : 128 * (jB + 1)],
                    rhs=qt[64:128, 512 * qc + oB : 512 * qc + oB + nB],
                    start=True, stop=True, tile_position=(64, 0),
                )

            for jp in range((n_k - 4) // 2):
                j = 2 * jp
                sp = ps.tile([128, 1024], F32, tag="sp")
                score_pair(j, 512, 0, j + 1, 512, 0, sp)
                pt = work.tile([128, 1024], BF16, tag="pt")
                nc.scalar.activation(pt, sp, mybir.ActivationFunctionType.Exp)
                for u in range(2):
                    nc.tensor.matmul(
                        acc, lhsT=vp[:, j + u, :],
                        rhs=pt[:, 512 * u : 512 * (u + 1)],
                        start=(j + u == 0), stop=False, skip_group_check=True,
                    )
                feeder()

            # diagonal group: block d masked below query sub-block d ->
            # stream cols 128d..512 (N=512-128d).
            d0 = n_k - 4
            spA = ps.tile([128, 1024], F32, tag="sp")
            score_pair(d0, 512, 0, d0 + 1, 384, 128, spA)
            feeder()
            spB = ps.tile([128, 1024], F32, tag="sp")
            score_pair(d0 + 2, 256, 256, d0 + 3, 128, 384, spB)
            ptA = work.tile([128, 1024], BF16, tag="pt")
            nc.scalar.activation(
                ptA[:, 0:896], spA[:, 0:896], mybir.ActivationFunctionType.Exp
            )
            ptB = work.tile([128, 1024], BF16, tag="pt")
            nc.scalar.activation(
                ptB[:, 0:256], spB[:, 0:256], mybir.ActivationFunctionType.Exp
            )
            nc.scalar.activation(
                ptB[:, 512:640], spB[:, 512:640], mybir.ActivationFunctionType.Exp
            )

            # mask the i==d query sub-block (first 128 cols of each trimmed
            # range): triangle (first half) / parity zero (second half)
            for pt_, off in [(ptA, 0), (ptA, 512), (ptB, 0), (ptB, 512)]:
                reg = pt_[:, off : off + 128]
                if first_half:
                    nc.vector.tensor_mul(reg, reg, tmt)
                else:
                    nc.vector.tensor_scalar_mul(reg, reg, hmtt)

            for d, (pt_, off, nd) in enumerate(
                [(ptA, 0, 512), (ptA, 512, 384), (ptB, 0, 256), (ptB, 512, 128)]
            ):
                j = d0 + d
                nc.tensor.matmul(
                    acc[:, 128 * d : 512],
                    lhsT=vp[:, j, :], rhs=pt_[:, off : off + nd],
                    start=(j == 0), stop=(j == n_k - 1), skip_group_check=True,
                )

            so = work.tile([H + 1, 512], F32, tag="so")
            nc.vector.tensor_copy(so, acc)
            nc.sync.dma_start(out=outp.ap()[:, qsl], in_=so)
            feeder()

        # ---- emission: proj(0), then attn(r) with proj(r+1) pieces woven
        # in between pair groups (feeder pops one piece every other call)
        st, items = proj_items(0)
        for it in items:
            it()
        for r in range(reps):
            nitems = []
            if r + 1 < reps:
                nst, nitems = proj_items(r + 1)
            pieces = list(nitems)
            tick = [0]

            def feeder():
                tick[0] += 1
                if tick[0] % 2 == 0 and pieces:
                    pieces.pop(0)()

            for qc in range(NQC):
                attn_chunk(st, qc, feeder)
            for it in pieces:
                it()
            if r + 1 < reps:
                st = nst

    nc.compile()
    return nc


# ------------------------------------------------------------------ host ----
def _perm_cols(h):
    blocks = list(range(h, NB, 2)) + list(range(1 - h, NB, 2))
    return np.concatenate([np.arange(128 * g, 128 * (g + 1)) for g in blocks])


def _to_bf16(a):
    import ml_dtypes

    return a.astype(ml_dtypes.bfloat16)


def _make_in_maps(batch_x, Wk, bk, Wq, bq, Wv):
    xT = np.ascontiguousarray(np.transpose(batch_x, (0, 2, 1)))
    wkv = np.ascontiguousarray(
        np.concatenate([Wk, Wv], axis=1).astype(np.float32)
    )
    wq_c = np.ascontiguousarray(Wq.astype(np.float32))
    bq8 = (bq.astype(np.float32) * 0.125).reshape(H, 1)
    bkv = np.concatenate(
        [bk.astype(np.float32), np.zeros(64, np.float32)]
    ).reshape(128, 1)
    tri = np.tril(np.ones((128, 128), np.float32)).T  # keep query>=key
    cols = {h: _perm_cols(h) for h in (0, 1)}
    wkv_b = _to_bf16(wkv)
    wq_b = _to_bf16(wq_c)
    tri_b = _to_bf16(np.ascontiguousarray(tri))
    return [
        {
            "xq": _to_bf16(np.ascontiguousarray(xT[b][:, cols[h]])),
            "wq": wq_b,
            "wkv": wkv_b,
            "bq8": bq8,
            "bkv": bkv,
            "hmt": np.full((128, 1), 1.0 - h, np.float32),
            "tmask": tri_b,
        }
        for b in range(B)
        for h in (0, 1)
    ]


def _combine(outps, bv):
    inv = {}
    for h in (0, 1):
        c = _perm_cols(h)
        inv[h] = np.empty_like(c)
        inv[h][c] = np.arange(T)
    out = np.empty((B, T, H), dtype=np.float32)
    for b in range(B):
        tot = np.zeros((H + 1, T), dtype=np.float64)
        for h in (0, 1):
            o = np.asarray(outps[2 * b + h], dtype=np.float64)
            tot += o[:, inv[h]]
        out[b] = (tot[0:H] / tot[H]).T + bv.astype(np.float64)
    return out


_CACHE = {}


def _get_nc():
    if "nc" not in _CACHE:
        _CACHE["nc"] = build(reps=1)
    return _CACHE["nc"]


def kernel(batch_x, Wk, bk, Wq, bq, Wv, bv):
    from concourse.bass_utils import run_bass_kernel_spmd

    batch_x = np.asarray(batch_x, dtype=np.float32)
    in_maps = _make_in_maps(
        batch_x, np.asarray(Wk), np.asarray(bk), np.asarray(Wq),
        np.asarray(bq), np.asarray(Wv),
    )
    nc = _get_nc()
    res = run_bass_kernel_spmd(nc, in_maps, core_ids=list(range(8)))
    outps = [res.results[c]["outp"] for c in range(8)]
    return _combine(outps, np.asarray(bv))


# revision 9
# speedup vs baseline: 1.2577x; 1.2577x over previous
"""Causal single-head attention (nn_AttentionHead) on 8 TRN2 NeuronCores.

Self-contained: kernel(**inputs) takes the full fp32 inputs and returns the
full [4, 4096, 64] output, distributing work across 8 cores internally.

Sharding: 8 cores = 4 batches x 2 key-parity shards. Core (b, h) computes,
for ALL 4096 queries of batch b (in parity-permuted order), unnormalized
flash-attention partials (numerator [64] + denominator) over the keys in
128-row blocks of parity h; the host sums the two partials per batch and
normalizes (softmax without max-subtraction is exact here; scores are O(1)).

v3 design (measured lineage: 84us -> 60.6us -> this):
  - bf16 datapath: x, weights, K/V/Q, exp output all bf16 (fp32 PSUM accum).
  - Explicit tile_position packing (measured ~1.9x on HW):
      * score matmuls: K=64 pairs row-tiled at (0,0)/(64,0)
      * q-pass: M=64 chunk pairs col-tiled at (0,0)/(0,64), single PSUM bank
  - Causal trimming: near-diagonal score/PV matmuls stream only the
    not-fully-masked query range; per-diagonal-block [128,128] masks
    (triangle first half, parity scalar second half).
  - V to natural layout via xbar DMA transpose (contiguous dest + DVE
    interleave copy).
  - Cross-rep software pipelining: the projection work (kv/q/vp) of rep r+1
    is emitted interleaved into the attention chunks of rep r, so the ACT
    engine (exp) never drains and PE stays dense. All rep-state double
    buffered (xqt, kvt, ktd, qt, vp).
  - exp on ACT only; PSUM->SBUF copies, biases, masks on DVE.
"""

import os
import sys
from contextlib import ExitStack

import numpy as np

for _p in ("/root/.axon_site/_ro/trn_rl_repo", "/opt/trn_rl_repo"):
    if os.path.isdir(_p) and _p not in sys.path:
        sys.path.append(_p)

import concourse.bacc as bacc
import concourse.tile as tile
from concourse import mybir

F32 = mybir.dt.float32
BF16 = mybir.dt.bfloat16

B, T, C, H = 4, 4096, 1024, 64
KT = C // 128  # contraction tiles
NKEY = T // 2  # keys per core
NKT = NKEY // 128  # key tiles per core
NQC = T // 512  # query chunks
NB = T // 128  # 128-row blocks


# ---------------------------------------------------------------- device ----
def build(reps: int = 1):
    nc = bacc.Bacc("TRN2", target_bir_lowering=False, debug=False)

    xq = nc.dram_tensor("xq", [C, T], BF16, kind="ExternalInput")
    wq = nc.dram_tensor("wq", [C, H], BF16, kind="ExternalInput")
    wkv = nc.dram_tensor("wkv", [C, 128], BF16, kind="ExternalInput")  # [Wk|Wv]
    bq8 = nc.dram_tensor("bq8", [H, 1], F32, kind="ExternalInput")  # bq/8
    bkv = nc.dram_tensor("bkv", [128, 1], F32, kind="ExternalInput")  # [bk;0]
    hmt = nc.dram_tensor("hmt", [128, 1], F32, kind="ExternalInput")  # 1-h
    tmask = nc.dram_tensor("tmask", [128, 128], BF16, kind="ExternalInput")

    outp = nc.dram_tensor("outp", [H + 1, T], F32, kind="ExternalOutput")

    with tile.TileContext(nc) as tc, ExitStack() as ctx:
        const = ctx.enter_context(tc.tile_pool(name="const", bufs=1))
        big = ctx.enter_context(tc.tile_pool(name="big", bufs=2))
        kvq = ctx.enter_context(tc.tile_pool(name="kvq", bufs=2))
        work = ctx.enter_context(tc.tile_pool(name="work", bufs=3))
        ps = ctx.enter_context(tc.tile_pool(name="ps", bufs=2, space="PSUM"))
        ps1 = ctx.enter_context(tc.tile_pool(name="ps1", bufs=1, space="PSUM"))

        wqt = const.tile([128, KT, H], BF16)
        nc.sync.dma_start(out=wqt, in_=wq.ap().rearrange("(k p) m -> p k m", p=128))
        wkvt = const.tile([128, KT, 128], BF16)
        nc.sync.dma_start(out=wkvt, in_=wkv.ap().rearrange("(k p) m -> p k m", p=128))
        bq8t = const.tile([H, 1], F32)
        nc.sync.dma_start(out=bq8t, in_=bq8.ap())
        bkvt = const.tile([128, 1], F32)
        nc.sync.dma_start(out=bkvt, in_=bkv.ap())
        hmtt = const.tile([128, 1], F32)
        nc.sync.dma_start(out=hmtt, in_=hmt.ap())
        tmt = const.tile([128, 128], BF16)
        nc.sync.dma_start(out=tmt, in_=tmask.ap())

        consts = (wqt, wkvt, bq8t, bkvt, hmtt, tmt)

        # rep-state tiles are pool-cycled (bufs=2); proj_items(r) emits the
        # work that builds rep r's state, attn_chunk(r, st, qc) consumes it.
        state = {}

        def proj_items(r):
            """Generator of emission thunks for rep r's projections."""
            xqt = big.tile([128, KT, T], BF16, tag="xqt")
            kvt = kvq.tile([128, NKEY], BF16, tag="kvt")
            ktd = kvq.tile([128, NKEY], BF16, tag="ktd")
            qt = kvq.tile([128, T], BF16, tag="qt")
            vp = kvq.tile([128, NKT, H + 1], BF16, tag="vp")
            vscr = kvq.tile([128, NKT, H], BF16, tag="vscr")
            st = (xqt, kvt, ktd, qt, vp)
            xq_r = xq.ap().rearrange("(k p) t -> p k t", p=128)

            def dma_x():
                for i in range(4):
                    sl = slice(1024 * i, 1024 * (i + 1))
                    nc.sync.dma_start(out=xqt[:, :, sl], in_=xq_r[:, :, sl])

            def kv_chunk(kc):
                def f():
                    sl = slice(512 * kc, 512 * (kc + 1))
                    pkv = ps1.tile([128, 1024], F32, tag="proj")
                    for k in range(KT):
                        nc.tensor.matmul(
                            pkv[:, 0:512], lhsT=wkvt[:, k, :], rhs=xqt[:, k, sl],
                            start=(k == 0), stop=(k == KT - 1),
                        )
                    nc.vector.tensor_scalar_add(kvt[:, sl], pkv[:, 0:512], bkvt)
                    nc.sync.dma_start(out=ktd[64:128, sl], in_=kvt[0:64, sl])
                return f

            def vtrans():
                nc.sync.dma_start_transpose(vscr, kvt[64:128, :])
                nc.vector.tensor_copy(vp[:, :, 0:H], vscr)
                nc.vector.memset(vp[:, :, H], 1.0)

            def q_pair(qp):
                def f():
                    slA = slice(1024 * qp, 1024 * qp + 512)
                    slB = slice(1024 * qp + 512, 1024 * qp + 1024)
                    pq = ps1.tile([128, 1024], F32, tag="proj")
                    for k in range(KT):
                        nc.tensor.matmul(
                            pq[0:64, 0:512], lhsT=wqt[:, k, :], rhs=xqt[:, k, slA],
                            start=(k == 0), stop=(k == KT - 1),
                            tile_position=(0, 0), skip_group_check=True,
                        )
                        nc.tensor.matmul(
                            pq[64:128, 512:1024], lhsT=wqt[:, k, :],
                            rhs=xqt[:, k, slB],
                            start=(k == 0), stop=(k == KT - 1),
                            tile_position=(0, 64), skip_group_check=True,
                        )
                    nc.vector.tensor_scalar(
                        qt[0:H, slA], pq[0:64, 0:512], 0.125, bq8t,
                        op0=mybir.AluOpType.mult, op1=mybir.AluOpType.add,
                    )
                    nc.vector.tensor_scalar(
                        qt[0:H, slB], pq[64:128, 512:1024], 0.125, bq8t,
                        op0=mybir.AluOpType.mult, op1=mybir.AluOpType.add,
                    )
                    nc.sync.dma_start(
                        out=qt[64:128, 1024 * qp : 1024 * qp + 1024],
                        in_=qt[0:64, 1024 * qp : 1024 * qp + 1024],
                    )
                return f

            items = [dma_x]
            items += [kv_chunk(kc) for kc in range(4)]
            items.append(vtrans)
            items += [q_pair(qp) for qp in range(4)]
            return st, items

        def attn_chunk(st, qc, feeder):
            xqt, kvt, ktd, qt, vp = st
            qsl = slice(512 * qc, 512 * (qc + 1))
            first_half = qc < NQC // 2
            cc = qc if first_half else qc - NQC // 2
            n_k = 4 * cc + 4
            acc = ps.tile([H + 1, 512], F32, tag="acc")

            def score_pair(jA, nA, oA, jB, nB, oB, sp):
                nc.tensor.matmul(
                    sp[:, 0:nA],
                    lhsT=kvt[0:64, 128 * jA : 128 * (jA + 1)],
                    rhs=qt[0:64, 512 * qc + oA : 512 * qc + oA + nA],
                    start=True, stop=True, tile_position=(0, 0),
                )
                nc.tensor.matmul(
                    sp[:, 512 : 512 + nB],
                    lhsT=ktd[64:128, 128 * jB : 128 * (jB + 1)],
                    rhs=qt[64:128, 512 * qc + oB : 512 * qc + oB + nB],
                    start=True, stop=True, tile_position=(64, 0),
                )

            for jp in range((n_k - 4) // 2):
                j = 2 * jp
                sp = ps.tile([128, 1024], F32, tag="sp")
                score_pair(j, 512, 0, j + 1, 512, 0, sp)
                pt = work.tile([128, 1024], BF16, tag="pt")
                nc.scalar.activation(pt, sp, mybir.ActivationFunctionType.Exp)
                for u in range(2):
                    nc.tensor.matmul(
                        acc, lhsT=vp[:, j + u, :],
                        rhs=pt[:, 512 * u : 512 * (u + 1)],
                        start=(j + u == 0), stop=False, skip_group_check=True,
                    )
                feeder()

            # diagonal group: block d masked below query sub-block d ->
            # stream cols 128d..512 (N=512-128d).
            d0 = n_k - 4
            spA = ps.tile([128, 1024], F32, tag="sp")
            score_pair(d0, 512, 0, d0 + 1, 384, 128, spA)
            feeder()
            spB = ps.tile([128, 1024], F32, tag="sp")
            score_pair(d0 + 2, 256, 256, d0 + 3, 128, 384, spB)
            ptA = work.tile([128, 1024], BF16, tag="pt")
            nc.scalar.activation(
                ptA[:, 0:896], spA[:, 0:896], mybir.ActivationFunctionType.Exp
            )
            ptB = work.tile([128, 1024], BF16, tag="pt")
            nc.scalar.activation(
                ptB[:, 0:256], spB[:, 0:256], mybir.ActivationFunctionType.Exp
            )
            nc.scalar.activation(
                ptB[:, 512:640], spB[:, 512:640], mybir.ActivationFunctionType.Exp
            )

            # mask the i==d query sub-block (first 128 cols of each trimmed
            # range): triangle (first half) / parity zero (second half)
            for pt_, off in [(ptA, 0), (ptA, 512), (ptB, 0), (ptB, 512)]:
                reg = pt_[:, off : off + 128]
                if first_half:
                    nc.vector.tensor_mul(reg, reg, tmt)
                else:
                    nc.vector.tensor_scalar_mul(reg, reg, hmtt)

            for d, (pt_, off, nd) in enumerate(
                [(ptA, 0, 512), (ptA, 512, 384), (ptB, 0, 256), (ptB, 512, 128)]
            ):
                j = d0 + d
                nc.tensor.matmul(
                    acc[:, 128 * d : 512],
                    lhsT=vp[:, j, :], rhs=pt_[:, off : off + nd],
                    start=(j == 0), stop=(j == n_k - 1), skip_group_check=True,
                )

            so = work.tile([H + 1, 512], F32, tag="so")
            nc.vector.tensor_copy(so, acc)
            nc.sync.dma_start(out=outp.ap()[:, qsl], in_=so)
            feeder()

        # ---- emission: proj(0), then attn(r) interleaved with proj(r+1):
        # one whole proj item woven in after each attention chunk
        st, items = proj_items(0)
        for it in items:
            it()
        for r in range(reps):
            nitems = []
            if r + 1 < reps:
                nst, nitems = proj_items(r + 1)
            pieces = list(nitems)

            def feeder():
                pass

            for qc in range(NQC):
                attn_chunk(st, qc, feeder)
                if qc < len(pieces):
                    pieces[qc]()
            for it in pieces[NQC:]:
                it()
            if r + 1 < reps:
                st = nst

    nc.compile()
    return nc


# ------------------------------------------------------------------ host ----
def _perm_cols(h):
    blocks = list(range(h, NB, 2)) + list(range(1 - h, NB, 2))
    return np.concatenate([np.arange(128 * g, 128 * (g + 1)) for g in blocks])


def _to_bf16(a):
    import ml_dtypes

    return a.astype(ml_dtypes.bfloat16)


def _make_in_maps(batch_x, Wk, bk, Wq, bq, Wv):
    xT = np.ascontiguousarray(np.transpose(batch_x, (0, 2, 1)))
    wkv = np.ascontiguousarray(
        np.concatenate([Wk, Wv], axis=1).astype(np.float32)
    )
    wq_c = np.ascontiguousarray(Wq.astype(np.float32))
    bq8 = (bq.astype(np.float32) * 0.125).reshape(H, 1)
    bkv = np.concatenate(
        [bk.astype(np.float32), np.zeros(64, np.float32)]
    ).reshape(128, 1)
    tri = np.tril(np.ones((128, 128), np.float32)).T  # keep query>=key
    cols = {h: _perm_cols(h) for h in (0, 1)}
    wkv_b = _to_bf16(wkv)
    wq_b = _to_bf16(wq_c)
    tri_b = _to_bf16(np.ascontiguousarray(tri))
    return [
        {
            "xq": _to_bf16(np.ascontiguousarray(xT[b][:, cols[h]])),
            "wq": wq_b,
            "wkv": wkv_b,
            "bq8": bq8,
            "bkv": bkv,
            "hmt": np.full((128, 1), 1.0 - h, np.float32),
            "tmask": tri_b,
        }
        for b in range(B)
        for h in (0, 1)
    ]


def _combine(outps, bv):
    inv = {}
    for h in (0, 1):
        c = _perm_cols(h)
        inv[h] = np.empty_like(c)
        inv[h][c] = np.arange(T)
    out = np.empty((B, T, H), dtype=np.float32)
    for b in range(B):
        tot = np.zeros((H + 1, T), dtype=np.float64)
        for h in (0, 1):
            o = np.asarray(outps[2 * b + h], dtype=np.float64)
            tot += o[:, inv[h]]
        out[b] = (tot[0:H] / tot[H]).T + bv.astype(np.float64)
    return out


_CACHE = {}


def _get_nc():
    if "nc" not in _CACHE:
        _CACHE["nc"] = build(reps=1)
    return _CACHE["nc"]


def kernel(batch_x, Wk, bk, Wq, bq, Wv, bv):
    from concourse.bass_utils import run_bass_kernel_spmd

    batch_x = np.asarray(batch_x, dtype=np.float32)
    in_maps = _make_in_maps(
        batch_x, np.asarray(Wk), np.asarray(bk), np.asarray(Wq),
        np.asarray(bq), np.asarray(Wv),
    )
    nc = _get_nc()
    res = run_bass_kernel_spmd(nc, in_maps, core_ids=list(range(8)))
    outps = [res.results[c]["outp"] for c in range(8)]
    return _combine(outps, np.asarray(bv))
